# revision 27
# baseline (speedup 1.0000x reference)
"""Trainium2 Bass kernel for nn_CrossAttentionBlock (B=4, C=512, H=W=64).

Decomposition across 8 NeuronCores: core = (batch b, query-half h).
All heavy matmuls run in bf16 (1 cycle/row on the PE vs 4 for fp32);
the harness tolerance (2e-2) leaves orders of magnitude of margin since
AdaIN only consumes aggregate statistics of the attention output.

Per core:
  prologue (streamed per 512-column block as DMA lands; x1 via the SP
  DMA queue, x0 via the Activation DMA queue so issue+transfer overlap):
    theta/phi = conv1x1(x1) packed as one 128-row projection -> bf16
    g         = conv1x1(x0) + g_b (folded: softmax-normalizing (g+b)
                equals normalizing g then adding b) -> bf16,
                PE-transposed into g_extT [keys, 65] with a ones column
  main loop (query-half outer, mi = key-chunk inner):
    fT = theta^T phi (PE); p = exp(fT) alternating between the ACT
    engine (true exp, bf16 out) and the DVE (bitcast fast-exp: bf16
    bit pattern = int16(f * 128*log2(e) + 127*128), ~3% sawtooth error
    that washes out in the AdaIN statistics);
    yT_ext += [g,1]^T p (PE) -> numerator rows 0..63, denominator row 64.
    Alternating exp engines keeps the PE the pacer at ~95% duty so the
    HAM clock gate holds 2.4 GHz (at ~75% duty it oscillates to 1.2).
  per query-half tail: transpose yT, scale by 1/denominator (batched
    reciprocal + broadcast multiply), AllGather the half (bf16) --
    the first half's collective hides under the second half's compute.
  tail: W_y consumed only as per-channel bn stats, final out = r*x0 + t
    split across the Pool engine and DVE, streamed output DMA.

SPMD uniformity: inputs are host-permuted so each core's queries and
output channels come first; the host un-permutes output columns. The
AllGather output is ordered [even core | odd core] = true token order
for both pair members, and key order is a contraction index (invariant).
"""
import numpy as np
from contextlib import ExitStack

import ml_dtypes

import concourse.bass as bass
import concourse.tile as tile
from concourse import mybir
from concourse.bass_utils import run_bass_kernel_spmd

FP32 = mybir.dt.float32
BF16 = mybir.dt.bfloat16
I16 = mybir.dt.int16
I32 = mybir.dt.int32
ALU = mybir.AluOpType
ACTF = mybir.ActivationFunctionType

B, C, H, W = 4, 512, 64, 64
N = H * W          # 4096 tokens
C8 = C // 8        # 64 inner channels
NH = N // 2        # 2048 queries per core
OC = C // 2        # 256 output channels per core
EPS = 1e-5

# fast-exp constants: bf16 bits of exp(x) ~= int16(x * 2^7*log2(e) + 127*2^7)
FEXP_A = 184.66296501
FEXP_B = 16256.0

REPLICA_PAIRS = [[0, 1], [2, 3], [4, 5], [6, 7]]

NPBF16 = ml_dtypes.bfloat16


def _split_excess_waits(nc, max_waits=1, drain_max=1):
    """walrus here rejects instructions carrying more than ~2 sync waits; move
    extras to preceding NoOps on the same engine (semantics preserved: waits
    run before the instruction, engine streams are sequential)."""
    for blk in nc.main_func.blocks:
        insts = blk.instructions
        k = 0
        while k < len(insts):
            inst = insts[k]
            si = inst.sync_info
            cap = drain_max if inst.opcode == "Drain" else max_waits
            if si is not None and si.on_wait and len(si.on_wait) > cap:
                waits = list(si.on_wait)
                keep = waits[-cap:]
                extra = waits[:-cap]
                pos = k
                for j in range(0, len(extra), cap):
                    nop = mybir.InstNoOp(name=f"{inst.name}-wsplit{j}", ins=[], outs=[])
                    nop.engine = inst.engine
                    nop.sync_info = mybir.SyncInfo(
                        on_wait=extra[j : j + cap], on_update=[]
                    )
                    insts.insert(pos, nop)
                    pos += 1
                    k += 1
                inst.sync_info = mybir.SyncInfo(on_wait=keep, on_update=list(si.on_update))
            k += 1


def build_nc():
    nc = bass.Bass()

    x0 = nc.dram_tensor("x0", [C, N], BF16, kind="ExternalInput")
    x1 = nc.dram_tensor("x1", [C, N], BF16, kind="ExternalInput")
    tp_wT = nc.dram_tensor("tp_wT", [C, 128], BF16, kind="ExternalInput")
    tp_b = nc.dram_tensor("tp_b", [128, 1], FP32, kind="ExternalInput")
    g_wT = nc.dram_tensor("g_wT", [C, C8], BF16, kind="ExternalInput")
    g_b64 = nc.dram_tensor("g_b64", [C8, 1], FP32, kind="ExternalInput")
    W_wTh = nc.dram_tensor("W_wTh", [128, OC], BF16, kind="ExternalInput")
    W_bh = nc.dram_tensor("W_bh", [128, 2], FP32, kind="ExternalInput")
    ident = nc.dram_tensor("ident", [C8 + 1, C8 + 1], FP32, kind="ExternalInput")
    out = nc.dram_tensor("out", [OC, N], BF16, kind="ExternalOutput")

    y_bounce = [nc.dram_tensor(f"y_bounce{q}", [NH // 2, C8], BF16) for q in range(2)]
    y_gath = [nc.dram_tensor(f"y_gath{q}", [NH, C8], BF16) for q in range(2)]

    with tile.TileContext(nc) as tc, ExitStack() as ctx:
        wpool = ctx.enter_context(tc.tile_pool(name="weights", bufs=1))
        big = ctx.enter_context(tc.tile_pool(name="big", bufs=1))

        # ---- weights to SBUF (SP queue; tail-only weights via SWDGE) ----
        tp_w_sb = wpool.tile([128, 4, 128], BF16)
        nc.sync.dma_start(out=tp_w_sb[:], in_=tp_wT[:].rearrange("(c p) o -> p c o", c=4))
        g_w_sb = wpool.tile([128, 4, C8], BF16)
        nc.sync.dma_start(out=g_w_sb[:], in_=g_wT[:].rearrange("(c p) o -> p c o", c=4))
        tp_b_sb = wpool.tile([128, 1], FP32)
        nc.sync.dma_start(out=tp_b_sb[:], in_=tp_b[:])
        g_b_sb = wpool.tile([C8, 1], FP32)
        nc.sync.dma_start(out=g_b_sb[:], in_=g_b64[:])
        W_w_sb = wpool.tile([128, OC], BF16)
        nc.gpsimd.dma_start(out=W_w_sb[:], in_=W_wTh[:])
        W_b_sb = wpool.tile([128, 2], FP32)
        nc.gpsimd.dma_start(out=W_b_sb[:], in_=W_bh[:])
        id_sb = wpool.tile([C8 + 1, C8 + 1], FP32)
        nc.sync.dma_start(out=id_sb[:], in_=ident[:])
        twarm = wpool.tile([128, 1], FP32)
        nc.scalar.activation(twarm[:], tp_b_sb[:], ACTF.Exp)
        id_bf = wpool.tile([C8, C8], BF16)
        nc.vector.tensor_copy(id_bf[:], id_sb[0:C8, 0:C8])

        # ---- persistent big tensors ----
        x0_sb = big.tile([128, 4, N], BF16)      # c-chunk on middle index
        x1_sb = big.tile([128, 4, N], BF16)
        theta_sb = big.tile([128, N], BF16)      # keys; rows 64-127 stay zero
        phi_sb = big.tile([128, NH], BF16)       # queries; rows 64-127 duplicate 0-63
        g_extT = big.tile([128, 32, C8 + 1], BF16)  # [m-chunk, 65] per chunk
        yT_sb = big.tile([C8 + 1, NH], FP32)
        ys_sb = big.tile([128, 16, C8], BF16)    # normalized y, token-major
        yv_sb = big.tile([128, N], BF16)         # y view; rows 64-127 duplicate

        nc.gpsimd.memset(g_extT[:, :, C8:C8 + 1], 1.0)
        nc.gpsimd.memset(theta_sb[C8:128, :], 0.0)

        # ---- input DMA: x1 on the SP queue, x0 on the ACT queue (parallel
        # issue + transfer); near blocks first ----
        def load_block(eng, t_dram, t_sb, b):
            cols = slice(b * 512, (b + 1) * 512)
            eng.dma_start(
                out=t_sb[:, :, cols],
                in_=t_dram[:, cols].rearrange("(c p) n -> p c n", c=4))

        for b in range(8):
            load_block(nc.sync, x1, x1_sb, b)
        for b in range(8):
            load_block(nc.scalar, x0, x0_sb, b)

        # ---- prologue per block: theta/phi proj, g proj, g transposes ----
        ps_big = ctx.enter_context(tc.tile_pool(name="ps_big", bufs=3, space="PSUM"))
        gstage = ctx.enter_context(tc.tile_pool(name="gstage", bufs=4))

        def proj_block(b, on_act=False):
            # on_act: route the elementwise psum->sbuf moves through the ACT
            # engine so mid-main-loop insertions do not delay DVE fast-exps.
            cols = slice(b * 512, (b + 1) * 512)
            pp = ps_big.tile([128, 1024], FP32, tag="ps", name="pp")
            for c in range(4):
                nc.tensor.matmul(pp[:, 0:512], tp_w_sb[:, c, :],
                                 x1_sb[:, c, cols], start=(c == 0), stop=(c == 3))
            if on_act:
                nc.scalar.activation(theta_sb[0:C8, cols], pp[0:C8, 0:512],
                                     ACTF.Identity, bias=tp_b_sb[0:C8, :])
            else:
                nc.vector.tensor_scalar_add(theta_sb[0:C8, cols], pp[0:C8, 0:512],
                                            tp_b_sb[0:C8, :])
            if b < 4:
                nc.vector.tensor_scalar_add(phi_sb[0:C8, cols], pp[C8:128, 0:512],
                                            tp_b_sb[C8:128, :])
                nc.scalar.activation(phi_sb[C8:128, cols], pp[C8:128, 0:512],
                                     ACTF.Identity, bias=tp_b_sb[C8:128, :])
            # g projection (+ folded g_b)
            pg = ps_big.tile([128, 1024], FP32, tag="ps", name="pg")
            for c in range(4):
                nc.tensor.matmul(pg[0:C8, 0:512], g_w_sb[:, c, :],
                                 x0_sb[:, c, cols], start=(c == 0), stop=(c == 3))
            g_blk = gstage.tile([C8, 512], BF16, tag="g", name="g_blk")
            if on_act:
                nc.scalar.activation(g_blk[:], pg[0:C8, 0:512],
                                     ACTF.Identity, bias=g_b_sb[:])
            else:
                nc.vector.tensor_scalar_add(g_blk[:], pg[0:C8, 0:512], g_b_sb[:])
            # transpose 4 key-chunks of 128 into g_extT (bf16: 1 cyc/row)
            tr = ps_big.tile([128, 1024], FP32, tag="ps", name="tr")
            trv = tr[:].bitcast(BF16)
            for k in range(4):
                nc.tensor.transpose(trv[:, k * 64:(k + 1) * 64],
                                    g_blk[:, k * 128:(k + 1) * 128],
                                    id_bf[:])
            nc.vector.tensor_copy(g_extT[:, 4 * b:4 * b + 4, 0:C8], trv[:, 0:256])

        # ---- x0 instance stats (own channels = chunks 0, 1) ----
        stat = ctx.enter_context(tc.tile_pool(name="stats", bufs=1))
        xst = [stat.tile([128, 8, 6], FP32, tag=f"xst{oc}", name=f"xst{oc}")
               for oc in range(2)]

        def x0_stats_block(b):
            for oc in range(2):
                nc.vector.bn_stats(xst[oc][:, b, :],
                                   x0_sb[:, oc, b * 512:(b + 1) * 512])

        proj_block(0)
        proj_block(1)

        # ---- main attention loop (query-half outer, mi inner) ----
        pend = {6: 2, 10: 3, 14: 4, 18: 5, 22: 6, 26: 7}
        stats_pend = {2: 2, 6: 3, 10: 4, 14: 5, 18: 6, 22: 7, 26: 0, 30: 1}
        ps_y = ctx.enter_context(tc.tile_pool(name="ps_y", bufs=1, space="PSUM"))
        ptpool = ctx.enter_context(tc.tile_pool(name="pT", bufs=6))
        ystage = ctx.enter_context(tc.tile_pool(name="ystage", bufs=2))
        for q in range(2):
            py = ps_y.tile([C8 + 1, 1024], FP32, tag="py", name="py")
            for mi in range(32):
                if q == 0 and mi in pend:
                    proj_block(pend[mi], on_act=(pend[mi] % 2 == 0))
                if q == 1 and mi in stats_pend:
                    x0_stats_block(stats_pend[mi])
                ft = ps_big.tile([128, 1024], FP32, tag="ps", name="ft")
                for s in range(2):
                    nc.tensor.matmul(
                        ft[:, s * 512:(s + 1) * 512],
                        theta_sb[:, mi * 128:(mi + 1) * 128],
                        phi_sb[:, q * 1024 + s * 512: q * 1024 + (s + 1) * 512],
                        start=True, stop=True)
                pt = ptpool.tile([128, 1024], BF16, tag="pt", name="pt")
                nc.scalar.activation(pt[:, 0:512], ft[:, 0:512], ACTF.Exp)
                nc.vector.tensor_scalar(pt[:, 512:1024].bitcast(I16),
                                        ft[:, 512:1024],
                                        FEXP_A, FEXP_B, ALU.mult, ALU.add)
                for s in range(2):
                    nc.tensor.matmul(
                        py[:, s * 512:(s + 1) * 512],
                        g_extT[:, mi, :],
                        pt[:, s * 512:(s + 1) * 512],
                        start=(mi == 0), stop=(mi == 31))

            # ---- per-half: transpose + normalize + gather ----
            qc = slice(q * 1024, (q + 1) * 1024)
            nc.vector.tensor_copy(yT_sb[:, qc], py[:])
            for t2 in range(2):
                ptile = ps_big.tile([128, 1024], FP32, tag="ps", name="ptile")
                ptv = ptile[:].rearrange("p (j e) -> p j e", j=4)
                for j4 in range(4):
                    j = 8 * q + 4 * t2 + j4
                    nc.tensor.transpose(ptv[:, j4, 0:C8 + 1],
                                        yT_sb[:, j * 128:(j + 1) * 128], id_sb[:])
                rec = ystage.tile([128, 4, 1], FP32, tag="rec", name="rec")
                nc.vector.reciprocal(rec[:], ptv[:, :, C8:C8 + 1])
                nc.vector.tensor_tensor(
                    ys_sb[:, 8 * q + 4 * t2: 8 * q + 4 * t2 + 4, :],
                    ptv[:, :, 0:C8],
                    rec[:].broadcast_to([128, 4, C8]),
                    ALU.mult)
            nc.sync.dma_start(
                out=y_bounce[q][:].rearrange("(j p) w -> p j w", p=128),
                in_=ys_sb[:, 8 * q: 8 * q + 8, :])
            nc.gpsimd.collective_compute(
                "AllGather", ALU.bypass,
                replica_groups=REPLICA_PAIRS,
                ins=[y_bounce[q][:]],
                outs=[y_gath[q][:]],
            )

        # ---- assemble yv from the two gathers ----
        for (a0, q, r0) in [(0, 0, 0), (16, 1, 0), (32, 0, 1024), (48, 1, 1024)]:
            src = y_gath[q][r0:r0 + 1024, :].rearrange("(a b) w -> a (b w)", a=16)
            nc.sync.dma_start(out=yv_sb[a0:a0 + 16, :], in_=src)
            nc.sync.dma_start(out=yv_sb[64 + a0:64 + a0 + 16, :], in_=src)

        xagg2 = stat.tile([128, 2, 2], FP32, tag="xagg2", name="xagg2")
        for oc in range(2):
            nc.vector.bn_aggr(xagg2[:, oc, :], xst[oc][:])

        # ---- phase 2: W_y stats + per-channel affine + output ----
        with tc.tile_pool(name="sc", bufs=1) as sc, \
             tc.tile_pool(name="outp", bufs=4) as outp:
            # PE warm-up fodder while the collective lands (reads the first
            # yv chunk so it can't start before data exists)
            for _ in range(8):
                dm = ps_big.tile([128, 1024], FP32, tag="ps", name="dm")
                nc.tensor.matmul(dm[:, 0:512], W_w_sb[0:16, 0:128],
                                 yv_sb[0:16, 0:512], start=True, stop=True)

            # rowmean of yv (rows 64-127 duplicate rows 0-63; W_w rows there
            # are zero so the duplicate contributes nothing to the matmul)
            yvst = sc.tile([128, 8, 6], FP32, tag="yvst", name="yvst")
            for mb in range(8):
                nc.vector.bn_stats(yvst[:, mb, :],
                                   yv_sb[:, mb * 512:(mb + 1) * 512])
            yv_agg = sc.tile([128, 2], FP32, tag="yv_agg", name="yv_agg")
            nc.vector.bn_aggr(yv_agg[:], yvst[:])
            mean_bf = sc.tile([128, 1], BF16, tag="mean_bf", name="mean_bf")
            nc.vector.tensor_copy(mean_bf[:], yv_agg[:, 0:1])

            sq_acc = sc.tile([128, 2, 4], FP32, tag="sq_acc", name="sq_acc")
            scr = sc.tile([128, 1024], FP32, tag="scr", name="scr", bufs=2)
            for oc in range(2):
                for mb in range(4):
                    pw = ps_big.tile([128, 1024], FP32, tag="ps", name="pw")
                    for t in range(2):
                        nc.tensor.matmul(
                            pw[:, t * 512:(t + 1) * 512],
                            W_w_sb[:, oc * 128:(oc + 1) * 128],
                            yv_sb[:, mb * 1024 + t * 512: mb * 1024 + (t + 1) * 512],
                            start=True, stop=True)
                    nc.scalar.activation(scr[:], pw[:], ACTF.Square,
                                         accum_out=sq_acc[:, oc, mb:mb + 1])

            pmw = ps_big.tile([128, 1024], FP32, tag="ps", name="pmw")
            for oc in range(2):
                nc.tensor.matmul(pmw[:, oc:oc + 1],
                                 W_w_sb[:, oc * 128:(oc + 1) * 128],
                                 mean_bf[:], start=True, stop=True)

            # scalar chain, all [128, 2] (one column per oc half); each oc
            # half of W_y has N columns
            mean0 = sc.tile([128, 2], FP32, tag="mean0", name="mean0")
            nc.vector.tensor_copy(mean0[:], pmw[:, 0:2])
            sx2 = sc.tile([128, 2], FP32, tag="sx2", name="sx2")
            for oc in range(2):
                nc.vector.tensor_reduce(sx2[:, oc:oc + 1], sq_acc[:, oc, :],
                                        mybir.AxisListType.X, ALU.add)
            m2 = sc.tile([128, 2], FP32, tag="m2", name="m2")
            nc.vector.tensor_mul(m2[:], mean0[:], mean0[:])
            vs = sc.tile([128, 2], FP32, tag="vs", name="vs")
            nc.vector.tensor_scalar(vs[:], sx2[:], 1.0 / N, EPS,
                                    ALU.mult, ALU.add)
            nc.vector.tensor_sub(vs[:], vs[:], m2[:])
            vc = sc.tile([128, 2], FP32, tag="vc", name="vc")
            nc.vector.tensor_scalar_add(vc[:], xagg2[:, :, 1], EPS)
            rc = sc.tile([128, 2], FP32, tag="rc", name="rc")
            nc.vector.reciprocal(rc[:], vc[:])
            ratio = sc.tile([128, 2], FP32, tag="ratio", name="ratio")
            nc.vector.tensor_mul(ratio[:], vs[:], rc[:])
            # rr = sqrt(ratio) = ratio * rsqrt(ratio), Newton x2 from the
            # bit-trick seed (avoids the sqrt activation-table reload)
            y0 = sc.tile([128, 2], FP32, tag="y0", name="y0")
            nc.vector.tensor_scalar(y0[:].bitcast(I32), ratio[:].bitcast(I32),
                                    1, None, ALU.arith_shift_right)
            nc.vector.tensor_scalar(y0[:].bitcast(I32), y0[:].bitcast(I32),
                                    -1, 0x5F3759DF, ALU.mult, ALU.add)
            hf = sc.tile([128, 2], FP32, tag="hf", name="hf")
            nc.vector.tensor_scalar_mul(hf[:], ratio[:], 0.5)
            t1 = sc.tile([128, 2], FP32, tag="t1", name="t1")
            for _ in range(2):
                nc.vector.tensor_mul(t1[:], y0[:], y0[:])
                nc.vector.tensor_mul(t1[:], t1[:], hf[:])
                nc.vector.tensor_scalar(t1[:], t1[:], -1.0, 1.5, ALU.mult, ALU.add)
                nc.vector.tensor_mul(y0[:], y0[:], t1[:])
            rr = sc.tile([128, 2], FP32, tag="rr", name="rr")
            nc.vector.tensor_mul(rr[:], ratio[:], y0[:])
            # t = (mean0 + W_b) - rr * mu_c
            mus = sc.tile([128, 2], FP32, tag="mus", name="mus")
            nc.vector.tensor_add(mus[:], mean0[:], W_b_sb[:])
            rmc = sc.tile([128, 2], FP32, tag="rmc", name="rmc")
            nc.vector.tensor_mul(rmc[:], rr[:], xagg2[:, :, 0])
            tt = sc.tile([128, 2], FP32, tag="tt", name="tt")
            nc.vector.tensor_sub(tt[:], mus[:], rmc[:])

            for mb in range(4):
                for oc in range(2):
                    cols = slice(mb * 1024, (mb + 1) * 1024)
                    ot = outp.tile([128, 1024], BF16, tag="ot", name="ot")
                    eng = nc.gpsimd if oc == 0 else nc.vector
                    eng.tensor_scalar(ot[:], x0_sb[:, oc, cols],
                                      rr[:, oc:oc + 1], tt[:, oc:oc + 1],
                                      ALU.mult, ALU.add)
                    nc.sync.dma_start(out=out[oc * 128:(oc + 1) * 128, cols], in_=ot[:])

    _split_excess_waits(nc)
    return nc


_NC_CACHE = None


def _get_nc():
    global _NC_CACHE
    if _NC_CACHE is None:
        _NC_CACHE = build_nc()
    return _NC_CACHE


def _core_inputs(x0f, x1f, tp_wT, tp_b, g_wT, g_b, W_wT, W_b, ident, core):
    b, half = core // 2, core % 2
    x0b, x1b = x0f[b], x1f[b]
    if half == 0:
        x0p = x0b
        x1p = x1b
        g_wp = g_wT
    else:
        # queries-first column permutation; own-channels-first row permutation
        x1p = np.concatenate([x1b[:, NH:], x1b[:, :NH]], axis=1)
        x0r = np.concatenate([x0b[OC:], x0b[:OC]], axis=0)
        x0p = np.concatenate([x0r[:, NH:], x0r[:, :NH]], axis=1)
        g_wp = np.concatenate([g_wT[OC:], g_wT[:OC]], axis=0)
    return {
        "x0": np.ascontiguousarray(x0p),
        "x1": np.ascontiguousarray(x1p),
        "tp_wT": tp_wT,
        "tp_b": tp_b,
        "g_wT": np.ascontiguousarray(g_wp),
        "g_b64": g_b,
        "W_wTh": np.ascontiguousarray(W_wT[:, half * OC:(half + 1) * OC]),
        "W_bh": np.ascontiguousarray(
            W_b[half * OC:(half + 1) * OC].reshape(2, 128).T),
        "ident": ident,
    }


def _prepare_in_maps(x0, x1, g_w, g_b, theta_w, theta_b, phi_w, phi_b, W_w, W_b):
    x0f = np.asarray(x0, np.float32).reshape(B, C, N).astype(NPBF16)
    x1f = np.asarray(x1, np.float32).reshape(B, C, N).astype(NPBF16)
    tp_wT = np.ascontiguousarray(
        np.concatenate([theta_w, phi_w], axis=0).T.astype(NPBF16))
    tp_b = np.ascontiguousarray(
        np.concatenate([theta_b, phi_b]).astype(np.float32)[:, None])
    g_wT = np.ascontiguousarray(np.asarray(g_w, np.float32).T.astype(NPBF16))
    W_wT = np.asarray(W_w, np.float32).T.astype(NPBF16)
    W_wT = np.ascontiguousarray(
        np.concatenate([W_wT, np.zeros_like(W_wT)], axis=0))
    ident = np.eye(C8 + 1, dtype=np.float32)
    g_b = np.ascontiguousarray(np.asarray(g_b, np.float32)[:, None])
    W_b = np.asarray(W_b, np.float32)
    return [
        _core_inputs(x0f, x1f, tp_wT, tp_b, g_wT, g_b, W_wT, W_b, ident, core)
        for core in range(8)
    ]


def kernel(x0, x1, g_w, g_b, theta_w, theta_b, phi_w, phi_b, W_w, W_b):
    in_maps = _prepare_in_maps(x0, x1, g_w, g_b, theta_w, theta_b,
                               phi_w, phi_b, W_w, W_b)
    nc = _get_nc()
    res = run_bass_kernel_spmd(nc, in_maps, core_ids=list(range(8)))

    out = np.empty((B, C, N), dtype=np.float32)
    for core in range(8):
        b, half = core // 2, core % 2
        o = np.asarray(res.results[core]["out"], np.float32)
        if half == 1:
            o = np.concatenate([o[:, NH:], o[:, :NH]], axis=1)
        out[b, half * OC:(half + 1) * OC] = o
    return out.reshape(B, C, H, W)
